# revision 47
# baseline (speedup 1.0000x reference)
"""LocalRCT sparse-attention kernel for 8 Trainium2 NeuronCores.

Full inputs in, full output out. Sharding: core = (batch b = core//2,
row-half h = core%2). Each core processes feature[b, :, 256h:256h+256, :]
(8 strip-rows of 16 tiles = 128 tiles of 32x32 px) and computes the two
tiny conv blocks on its p_low slice on-device.

v2 design notes (vs the fp32 baseline):
- All matmuls run in bf16 (1 cycle/row on the PE vs 4 for fp32).
- conv2-r writes chunk tiles o2d[128, l, 9, 17] whose partitions are the
  feature channel r (duplicated on both partition halves), so the QK
  matmul's weight-load AP gathers the per-tile keys directly from the
  conv output -- no rk scatter DMAs at all.
- exp() output is bf16; the AV matmul consumes it directly.
- The final normalize writes a compact [128, 384] tile per strip that is
  DMA'd densely to DRAM; the host inverts the layout permutation.
"""

import os
import sys

sys.path.insert(0, "/opt/trn_rl_repo")

import ml_dtypes
import numpy as np

import concourse.bacc as bacc
import concourse.bass as bass
import concourse.tile as tile
from concourse import mybir
from concourse.bass_utils import run_bass_kernel_spmd

F32 = mybir.dt.float32
BF16 = mybir.dt.bfloat16
AF = mybir.ActivationFunctionType
NPBF16 = ml_dtypes.bfloat16

RF, NLF, MG, FUSION = 64, 16, 16, 64
NROW = 8  # strip rows per core


def build_program():
    nc = bacc.Bacc("TRN2", target_bir_lowering=False, debug=False)

    feat = nc.dram_tensor("feat", [8, 128, 8192], BF16, kind="ExternalInput")
    pshard = nc.dram_tensor("pshard", [64, 13, 21], BF16, kind="ExternalInput")
    w1rT = nc.dram_tensor("w1rT", [64, 9, 64], BF16, kind="ExternalInput")
    b1r = nc.dram_tensor("b1r", [64, 2], F32, kind="ExternalInput")
    w2rd = nc.dram_tensor("w2rd", [64, 9, 2048], BF16, kind="ExternalInput")
    b2rd = nc.dram_tensor("b2rd", [128, 16], F32, kind="ExternalInput")
    w1tT = nc.dram_tensor("w1tT", [64, 9, 64], BF16, kind="ExternalInput")
    b1t = nc.dram_tensor("b1t", [64, 2], F32, kind="ExternalInput")
    w2tT = nc.dram_tensor("w2tT", [64, 9, 48], BF16, kind="ExternalInput")
    b2t = nc.dram_tensor("b2t", [48, 1], F32, kind="ExternalInput")
    hmask = nc.dram_tensor("hmask", [64, 11, 19], BF16, kind="ExternalInput")
    yout = nc.dram_tensor("yout", [8, 128, 384], F32, kind="ExternalOutput")

    with tile.TileContext(nc) as tc:
        with (
            tc.tile_pool(name="singles", bufs=1) as singles,
            tc.tile_pool(name="strips", bufs=4) as strips,
            tc.tile_pool(name="exps", bufs=6) as exps,
            tc.tile_pool(name="work", bufs=4) as work,
            tc.tile_pool(name="qkp", bufs=2, space="PSUM") as qkp,
            tc.tile_pool(name="avp", bufs=2, space="PSUM") as avp,
        ):
            # ---------------- setup: load weights & p_low ----------------
            w1rT_s = singles.tile([64, 9, 64], BF16)
            w2rd_s = singles.tile([64, 9, 2048], BF16)
            w1tT_s = singles.tile([64, 9, 64], BF16)
            w2tT_s = singles.tile([64, 9, 48], BF16)
            b1r_s = singles.tile([64, 2], F32)
            b2rd_s = singles.tile([128, 16], F32)
            b1t_s = singles.tile([64, 2], F32)
            b2t_s = singles.tile([48, 1], F32)
            p0 = singles.tile([64, 13, 21], BF16)
            hmask_s = singles.tile([64, 11, 19], BF16)
            for dst, src in [
                (p0, pshard), (w1tT_s, w1tT), (w2tT_s, w2tT), (b1t_s, b1t),
                (b2t_s, b2t), (hmask_s, hmask), (w1rT_s, w1rT),
                (b1r_s, b1r), (b2rd_s, b2rd), (w2rd_s, w2rd),
            ]:
                nc.sync.dma_start(out=dst[:], in_=src[:])

            # KSTAGE=-1 (bench only): skip all compute -- measures the
            # per-call launch floor.
            stage_all = int(os.environ.get("KSTAGE", "4"))

            # Pre-issue the first strip loads ahead of the conv/gather
            # section so they are not queued behind DMAs that wait on conv
            # semaphores (the SP ring is in-order).
            pre_strips = {}
            if stage_all >= 0:
                for ti in range(1):
                    s = strips.tile([128, 32, 8, 32], BF16, tag="strip")
                    nc.sync.dma_start(out=s[:], in_=feat[ti])
                    pre_strips[ti] = s

            # Dummy silu at t=0: pulls the Silu table-set load into the
            # DMA window instead of serializing before the first real silu.
            if stage_all >= 0:
                scr0 = work.tile([64, 1], F32, name="scr0")
                nc.scalar.activation(out=scr0[:], in_=p0[:, 0, 0:1],
                                     func=AF.Silu, scale=0.0)

            # ---------------- conv blocks ----------------
            # conv1: [64,13,21] -> [64,11,19] (valid), silu, ring mask
            def conv1(w1T_s, b1_s, nm):
                c1 = qkp.tile([64, 11, 19], F32, tag="qk_t")
                for k in range(9):
                    dy, dx = k // 3, k % 3
                    nc.tensor.matmul(
                        out=c1[:],
                        lhsT=w1T_s[:, k, :],
                        rhs=p0[:, dy:dy + 11, dx:dx + 19],
                        start=(k == 0), stop=(k == 8),
                    )
                h1 = singles.tile([64, 11, 19], BF16, name=nm)
                nc.scalar.activation(out=h1[:], in_=c1[:], func=AF.Silu,
                                     bias=b1_s[:, 0:1], scale=1.0)
                nc.gpsimd.tensor_tensor(h1[:], h1[:], hmask_s[:],
                                        op=mybir.AluOpType.mult)
                return h1

            if stage_all < 0:
                h1r = h1t = None
            else:
                h1t = conv1(w1tT_s, b1t_s, "h1t")
                h1r = conv1(w1rT_s, b1r_s, "h1r")

            # Prefetch the exp table set during ACT's idle window so the
            # loop's first exp pays no LoadActFuncSet.
            if stage_all >= 0:
                scr = work.tile([64, 1], F32, name="scr")
                nc.scalar.activation(out=scr[:], in_=b1r_s[:, 0:1],
                                     func=AF.Exp, scale=0.0)

            # Zero the block-diagonal key/value tiles up front -- these
            # depend on nothing and must not sit on the DVE queue between
            # the conv outputs and the rk copies (critical path).
            rk_t = [singles.tile([128, 128, 8], BF16, name=f"rk{i}")
                    for i in range(8)]
            tv_blk = singles.tile([128, 8, 64], BF16)
            if stage_all >= 0:
                for t_ in rk_t:
                    nc.vector.memset(t_[:], 0.0)
                nc.vector.memset(tv_blk[:], 0.0)


            # conv2-t: [48, 9, 17], partition = l*3 + ch (l-major)
            c2t = avp.tile([48, 9, 17], F32, tag="av0")
            for k in range(9 if stage_all >= 0 else 0):
                dy, dx = k // 3, k % 3
                nc.tensor.matmul(
                    out=c2t[:],
                    lhsT=w2tT_s[:, k, :],
                    rhs=h1t[:, dy:dy + 9, dx:dx + 17],
                    start=(k == 0), stop=(k == 8),
                )
            o2t = singles.tile([48, 9, 17], BF16)
            if stage_all >= 0:
                nc.scalar.activation(out=o2t[:], in_=c2t[:], func=AF.Identity,
                                     bias=b2t_s[:], scale=1.0)

            # conv2-r: for each l, a [128, 9, 17] chunk whose 128 weight
            # cols are channels {16r+l : r} duplicated on both halves, so
            # partition p = r (p<64) / r+64 (p>=64).
            o2d = singles.tile([128, 16, 9, 17], BF16)
            for l in range(16 if stage_all >= 0 else 0):
                c2 = avp.tile([128, 9, 17], F32, tag=f"av{l % 2}")
                for k in range(9):
                    dy, dx = k // 3, k % 3
                    nc.tensor.matmul(
                        out=c2[:],
                        lhsT=w2rd_s[:, k, 128 * l:128 * (l + 1)],
                        rhs=h1r[:, dy:dy + 9, dx:dx + 17],
                        start=(k == 0), stop=(k == 8),
                    )
                nc.vector.tensor_scalar_add(o2d[:, l], c2[:],
                                            b2rd_s[:, l:l + 1])

            # ---------------- materialize rk from o2d ----------------
            # rk_blk[64a + r, k*64 + pair]: k = (2di+dj)*16 + l,
            # pair = 8ti + jjp. Lane-aligned DVE copies (partitions = r on
            # both halves of o2d).
            # Block-diagonal: kk = 64a + (2di+dj)*16 + l; rows of the
            # other half stay zero so one full-width matmul computes both
            # tiles of a pair (same trick as tv_blk's zero padding).
            for ti in range(8 if stage_all >= 0 else 0):
                rk8 = rk_t[ti].rearrange(
                    "p (aa di dj l) jjp -> p aa di dj l jjp",
                    aa=2, di=2, dj=2, l=16)
                for a in range(2):
                    sp = 64 * a
                    for dj in range(2):
                        src = o2d[sp:sp + 64, :, ti:ti + 2,
                                  8 * a + dj:8 * a + dj + 8]
                        nc.vector.tensor_copy(
                            rk8[sp:sp + 64, a, :, dj, :, :],
                            src.transpose([0, 2, 1, 3]))

            # ---------------- gather tv ----------------
            # tv_blk[64a + 16c4 + l, s, pair]: s = 4a+ch (ch<3), s = 4a+3
            # is the softmax-denominator ones column. pair = 8ti + jjp.
            o2t5 = o2t.rearrange("(l ch) y x -> l ch y x", l=16, ch=3)
            for a in range(2 if stage_all >= 0 else 0):
                for di in range(2):
                    for dj in range(2):
                        c4 = 2 * di + dj
                        for ch in range(3):
                            nc.sync.dma_start(
                                out=tv_blk[64 * a + 16 * c4:
                                           64 * a + 16 * (c4 + 1),
                                           4 * a + ch, :],
                                in_=o2t5[:, ch, di:di + 8,
                                         8 * a + dj:8 * a + dj + 8])
                nc.vector.memset(tv_blk[64 * a:64 * (a + 1), 4 * a + 3, :], 1.0)

            # ---------------- main loop ----------------
            # KREPEAT>1 re-runs the hot loop in-NEFF (benchmarking only).
            # KSTAGE (bench only): 0=dma, 1=+qk, 2=+exp, 3=+av, 4=full.
            repeat = int(os.environ.get("KREPEAT", "1"))
            stage = stage_all
            if stage < 0:
                repeat = 0
            first_pass = set(pre_strips)
            for ti in [t for _ in range(repeat) for t in range(NROW)]:
                if ti in first_pass:
                    strip = pre_strips[ti]
                    first_pass.discard(ti)
                else:
                    strip = strips.tile([128, 32, 8, 32], BF16, tag="strip")
                    nc.sync.dma_start(out=strip[:], in_=feat[ti])

                mc = work.tile([128, 2, 32, 2, 3], F32, tag="mc")
                for g4 in range(2):
                    av_h = [avp.tile([128, 512], F32, name=f"avh{w}",
                                     tag=f"av{w}")
                            for w in range(2)]
                    for u in range(4):
                        jjp = 4 * g4 + u
                        w_, uu = u // 2, u % 2
                        if stage < 1:
                            continue
                        qk_t = qkp.tile([128, 1024], F32, tag="qk_t")
                        lhsT = rk_t[ti][:, :, jjp]
                        for v in range(2):
                            rhs = strip[:, 16 * v:16 * (v + 1), jjp, :]
                            nc.tensor.matmul(
                                out=qk_t[:, 512 * v:512 * (v + 1)],
                                lhsT=lhsT, rhs=rhs)
                        if stage < 2:
                            continue
                        exp_t = exps.tile([128, 1024], BF16, tag="exp_t")
                        nc.scalar.activation(out=exp_t[:], in_=qk_t[:],
                                             func=AF.Exp, scale=0.125)
                        if stage < 3:
                            continue
                        # AV+sum: out rows (4a+s) = (chA0..2,sumA,chB..,sumB)
                        lhsV = tv_blk[:, :, 8 * ti + jjp]
                        for v in range(2):
                            po = 32 * (2 * uu + v)
                            nc.tensor.matmul(
                                out=av_h[w_][po:po + 8, :],
                                lhsT=lhsV,
                                rhs=exp_t[:, 512 * v:512 * (v + 1)],
                                tile_position=(0, po))

                    if stage < 4:
                        continue
                    # normalize per column-half: each transpose can start as
                    # soon as its half's 4 AV matmuls are done
                    for w_ in range(2):
                        tt = work.tile([128, 512], F32, tag="tt")
                        nc.vector.transpose(out=tt[:], in_=av_h[w_][:])
                        t4 = tt.rearrange("p (cb r) -> p cb r", cb=16, r=32)[
                            :, :, 0:8].rearrange("p cb (aa s) -> p cb aa s",
                                                 aa=2, s=4)
                        rc = work.tile([128, 16, 2], F32, tag="rc")
                        nc.vector.reciprocal(out=rc[:], in_=t4[:, :, :, 3])
                        for s in range(3):
                            nc.vector.tensor_mul(
                                mc[:, g4, 16 * w_:16 * (w_ + 1), :, s],
                                t4[:, :, :, s], rc[:])
                    if stage >= 4:
                        # last strip's writes ride the (now idle) fast SP
                        # ring; earlier ones stay on SWDGE so they never
                        # block in-loop strip loads on the in-order SP queue
                        eng = nc.sync if ti == NROW - 1 else nc.gpsimd
                        eng.dma_start(
                            out=yout[ti][:, 192 * g4:192 * (g4 + 1)],
                            in_=mc[:, g4].rearrange("p b c d -> p (b c d)"))
    nc.compile()
    return nc


_PROGRAM_CACHE = {}


def _get_program():
    if "nc" not in _PROGRAM_CACHE:
        _PROGRAM_CACHE["nc"] = build_program()
    return _PROGRAM_CACHE["nc"]


def _prep_inputs(feature, p_low, r_w1, r_b1, r_gamma, r_beta, r_mean, r_var,
                 r_w2, r_b2, t_w1, t_b1, t_gamma, t_beta, t_mean, t_var,
                 t_w2, t_b2):
    f32 = np.float32

    def fold(w1, b1, g, be, m, v):
        s = (g / np.sqrt(v + f32(1e-5))).astype(f32)
        w1f = (w1 * s[:, None, None, None]).astype(f32)
        b1f = ((b1 - m) * s + be).astype(f32)
        return w1f, b1f

    def wT(w):  # [O,C,3,3] -> [C, 9, O]
        return np.ascontiguousarray(w.transpose(1, 2, 3, 0).reshape(
            w.shape[1], 9, w.shape[0]))

    w1rf, b1rf = fold(r_w1, r_b1, r_gamma, r_beta, r_mean, r_var)
    w1tf, b1tf = fold(t_w1, t_b1, t_gamma, t_beta, t_mean, t_var)

    # conv2-r duplicated chunks: col j of chunk l = channel 16*(j%64) + l
    w2rd = np.ascontiguousarray(
        r_w2.reshape(64, 16, 64, 3, 3).transpose(2, 3, 4, 1, 0).reshape(
            64, 9, 16, 64))            # [c, k, l, r]
    w2rd = np.concatenate([w2rd, w2rd], axis=3).reshape(64, 9, 2048)
    b2rd = np.concatenate([r_b2.reshape(64, 16)] * 2, axis=0).astype(f32)

    # conv2-t: col = l*3 + ch for original o = ch*16 + l
    w2t_lm = np.ascontiguousarray(
        t_w2.reshape(3, 16, 64, 3, 3).transpose(2, 3, 4, 1, 0).reshape(
            64, 9, 48))
    b2t_lm = np.ascontiguousarray(
        t_b2.reshape(3, 16).T.reshape(48, 1)).astype(f32)

    common = {
        "w1rT": wT(w1rf).astype(NPBF16),
        "b1r": np.stack([b1rf, -b1rf], 1).astype(f32),
        "w2rd": w2rd.astype(NPBF16), "b2rd": b2rd,
        "w1tT": wT(w1tf).astype(NPBF16),
        "b1t": np.stack([b1tf, -b1tf], 1).astype(f32),
        "w2tT": w2t_lm.astype(NPBF16), "b2t": b2t_lm,
    }

    # p_low padded: [B, 64, 17, 17] -> per (b,h): [64, 13, 21]
    ppad = np.zeros((p_low.shape[0], 64, 17 + 8, 21), f32)
    ppad[:, :, 2:19, 2:19] = p_low
    fbf = np.asarray(feature, f32).astype(NPBF16)
    in_maps = []
    for core in range(8):
        b, h = core // 2, core % 2
        m = dict(common)
        # [64, 256, 512] -> [ti, (half,ch), row, jjp, col] = [8, 128, 8192]
        fs = fbf[b, :, 256 * h:256 * (h + 1), :]
        fs = fs.reshape(64, 8, 32, 2, 8, 32).transpose(1, 3, 0, 2, 4, 5)
        m["feat"] = np.ascontiguousarray(fs).reshape(8, 128, 8192)
        m["pshard"] = np.ascontiguousarray(
            ppad[b, :, 8 * h:8 * h + 13, :]).astype(NPBF16)
        mk = np.zeros((11, 19), f32)
        for i in range(11):
            for j in range(19):
                gr, gc = 8 * h - 1 + i, j - 1
                if 0 <= gr <= 16 and 0 <= gc <= 16:
                    mk[i, j] = 1.0
        m["hmask"] = np.ascontiguousarray(
            np.broadcast_to(mk, (64, 11, 19))).astype(NPBF16)
        in_maps.append(m)
    return in_maps


def _decode_core(yraw):
    """yout [8, 128, 384] f32 -> [3, 256, 512] for one core."""
    a = np.asarray(yraw, np.float32).reshape(8, 2, 2, 32, 2, 2, 16, 2, 3)
    # dims: ti, uu, v, q, g4, w_, i, aa, s
    a = a.transpose(8, 0, 2, 6, 7, 4, 5, 1, 3)  # s, ti, v, i, aa, g4, w_, uu, q
    return np.ascontiguousarray(a).reshape(3, 256, 512)


def decode_outputs(per_core_yout):
    Y = np.zeros((4, 3, 512, 512), np.float32)
    for core in range(8):
        b, h = core // 2, core % 2
        Y[b, :, 256 * h:256 * (h + 1), :] = _decode_core(per_core_yout[core])
    return Y


def run(inputs, trace=False, tmpdir=None):
    nc = _get_program()
    in_maps = _prep_inputs(**{k: np.asarray(v) for k, v in inputs.items()})
    res = run_bass_kernel_spmd(nc, in_maps, core_ids=list(range(8)),
                               trace=trace, tmpdir=tmpdir)
    Y = decode_outputs([res.results[c]["yout"] for c in range(8)])
    return Y, res


def kernel(**inputs):
    return run(inputs)[0]
